# revision 83
# baseline (speedup 1.0000x reference)
"""GCN layer (gather -> aggregate -> @W -> bias -> PReLU) on 8 trn2 cores.

Strategy (v4: aggregate pre-projection features, project per dst window):
  - out[d] = prelu(dinv_d * (agg[d] @ W) + b), agg[d] = x'[d] + sum_{e: dst=d}
    x'[src_e], where x' = dinv[:,None] * x is precomputed on host (folds the
    src-side norm; self-loop term dinv_d^2 * x[d] @ W comes from the x'[d]
    row). Linearity lets us aggregate 256-dim inputs first and apply W once
    per dst, so no hs table is materialized on device.
  - 100000 nodes -> 102400 slots = 8 cores x 100 windows x 128; nodes are
    dealt to (core, window) by a per-window greedy that equalizes
    per-(window, src-quarter) in-edge counts across cores (the shared SPMD
    schedule pads every segment to the max count over cores, so balance is
    throughput). x' staged as 4 quarter tables [25600, 256] bf16 (identical
    on every core; gather idx fit int16); gather elem = 512B (mult-1 DMA).
  - Schedule layout is quarter-major: per q, the 13 window-groups'
    per-window max-count segments are concatenated with NO padding between
    groups (only each q column is padded to 128). 128-edge blocks may
    straddle a group boundary; their next-group matmuls are deferred
    ("pending") until that group's PSUM tiles open.
  - All gather idx streams and per-block rel columns are SBUF-resident,
    loaded in a few wide DMAs (>=512B per partition line dodges the 2x
    small-descriptor penalty). Each call's gather is split into two
    half-sized tiles for deep pipelining. Per block a one-hot S[e, d] =
    (iota_bank == rel[e]) is built on DVE (rel is dst offset from the
    block's first window, 0..255, bf16-exact); PE
    accumulates aggT_c[k, d] += G_c^T @ S per overlapped window (c = k-chunk
    of 2), G = dma_gather'ed x' rows. Self-loop rows enter each window's aggT
    via identity matmuls from SBUF-resident x'own slices (start=True only on
    the first chain per PSUM bank: start clears the whole bank's has_written
    bits).
  - Per window-pair: aggT bank copied wide to bf16 SBUF, per window projected
    out_psum[d, f] = sum_c aggT_c^T @ W_c. Epilogue: out = prelu(dinv_dst *
    psum + b) as max(u, alpha*u) when b == 0 and alpha uniform in [0,1]
    (true here); bf16 output written partition-major [128, NW*H], upcast +
    unpermuted on host.
"""
import sys
sys.path.insert(0, '/opt/trn_rl_repo')

import numpy as np
import ml_dtypes

N = 100000
NCORES = 8
SH = 12800                 # dst slots per core
NP = NCORES * SH           # 102400 slots
H = 128                    # output features
KIN = 256                  # input features
WIN = 128                  # dst window size
NW = SH // WIN             # 100 windows per core
WG = 8                     # windows per PSUM group
NG = (NW + WG - 1) // WG   # 13 groups (last has 4)
NQ = 4                     # source quarters (int16 gather idx: 25600 < 32768)
QTAB = NP // NQ            # 25600 rows per quarter table
bf16 = ml_dtypes.bfloat16


def _balance_nodes(e_dst, e_src):
    """Assign nodes to (core, window, slot) so that per-(window, quarter)
    in-edge counts are nearly equal across the 8 cores. Greedy on the true
    objective (growth of sum_q max_c load) + a bounded swap pass.
    Returns node_of_slot [NP] (-1 for pad slots)."""
    q = e_src // QTAB
    qd = np.bincount(e_dst * NQ + q, minlength=N * NQ).reshape(N, NQ)
    tot = qd.sum(1)
    order = np.argsort(-tot, kind='stable')

    node_of_slot = np.full(NP, -1, np.int64)
    per_w = N // NW
    assert per_w * NW == N and per_w <= NCORES * WIN
    for w in range(NW):
        chunk = order[w * per_w:(w + 1) * per_w]
        L = np.zeros((NCORES, NQ), np.int64)
        members = [[] for _ in range(NCORES)]
        for n in chunk:
            d = qd[n]
            Ld = L + d[None, :]
            mx = L.max(axis=0)
            cost = np.maximum(Ld - mx[None, :], 0).sum(1)
            full = np.array([len(m) >= WIN for m in members])
            c = int(np.argmin(cost + L.sum(1) * 1e-6 + full * (1 << 40)))
            members[c].append(n)
            L[c] += d
        # bounded swap pass among the lightest tail nodes
        for _ in range(6):
            improved = False
            for _try in range(400):
                mx = L.max(axis=0)
                obj = mx.sum()
                cs = int(np.argmax(L[:, np.argmax(mx - L.min(axis=0))]))
                best = None
                for n_i in range(max(0, len(members[cs]) - 6), len(members[cs])):
                    n = members[cs][n_i]
                    for ct in range(NCORES):
                        if ct == cs:
                            continue
                        for m_i in range(max(0, len(members[ct]) - 6), len(members[ct])):
                            m = members[ct][m_i]
                            L2 = L.copy()
                            L2[cs] += qd[m] - qd[n]
                            L2[ct] += qd[n] - qd[m]
                            no = L2.max(axis=0).sum()
                            if no < obj - 0.5:
                                best = (n_i, cs, m_i, ct)
                                obj = no
                if best is None:
                    break
                n_i, cs2, m_i, ct = best
                n, m = members[cs2][n_i], members[ct][m_i]
                members[cs2][n_i] = m
                members[ct][m_i] = n
                L[cs2] += qd[m] - qd[n]
                L[ct] += qd[n] - qd[m]
                improved = True
            if not improved:
                break
        for c in range(NCORES):
            for i, n in enumerate(members[c]):
                node_of_slot[c * SH + w * WIN + i] = n
    return node_of_slot


def _preprocess(edge_index):
    e_src = np.asarray(edge_index[0]).astype(np.int64)
    e_dst = np.asarray(edge_index[1]).astype(np.int64)

    deg = (np.bincount(e_dst, minlength=N) + 1).astype(np.float32)
    dinv = (1.0 / np.sqrt(deg)).astype(np.float32)

    node_of_slot = _balance_nodes(e_dst, e_src)
    real = node_of_slot >= 0
    slot_of_node = np.empty(N, np.int64)
    slot_of_node[node_of_slot[real]] = np.nonzero(real)[0]

    dinv_np = np.ones(NP, np.float32)           # indexed by SLOT
    dinv_np[real] = dinv[node_of_slot[real]]

    # real edges only; self-loops are applied from SBUF
    src = e_src
    dst = slot_of_node[e_dst]                   # dst position = slot
    E = src.shape[0]

    core = dst // SH
    w_all = (dst % SH) // WIN                # 0..NW-1
    g = w_all // WG
    q = src // QTAB                          # source quarter
    tab_row = src % QTAB                     # row within quarter table

    # sort by (core, q, window)  [quarter-major layout]
    key = (core * NQ + q) * NW + w_all
    nbins_pc = NQ * NW
    order = np.argsort(key, kind='stable')
    o_tab = tab_row[order]
    o_dst = dst[order]
    o_key = key[order]
    o_core = core[order]

    cnt_all = np.bincount(key, minlength=NCORES * nbins_pc)
    bin_start = np.concatenate([[0], np.cumsum(cnt_all)])[:-1]
    rank = np.arange(E, dtype=np.int64) - bin_start[o_key]   # within (c,q,w)

    # every window must have at least one real edge on SOME core, so the
    # shared schedule has a stop-flag anchor for each PSUM region
    wcnt = cnt_all.reshape(NCORES, NQ, NW).sum(axis=(0, 1))
    assert wcnt.min() >= 1

    # shared schedule: per (q,w) segment length = max count over cores; per
    # q column the segments are concatenated (window-major) and only the
    # column end is padded to a 128 multiple.
    maxcnt = cnt_all.reshape(NCORES, NQ, NW).max(axis=0)      # [NQ, NW]
    seg_end_in_col = np.cumsum(maxcnt, axis=1)                # [NQ, NW]
    wstart_in_col = seg_end_in_col - maxcnt
    collen = ((seg_end_in_col[:, -1] + WIN - 1) // WIN) * WIN  # [NQ]
    col_off = np.concatenate([[0], np.cumsum(collen)])
    TOT = int(col_off[-1])
    NBLK = TOT // WIN

    qw_in_core = o_key % nbins_pc
    q_in_core = qw_in_core // NW
    pos = (col_off[q_in_core] + wstart_in_col.reshape(-1)[qw_in_core] + rank)
    blkid = pos // WIN                                        # global block

    # per-block overlapped windows from the shared segment layout
    wmin_blk = np.zeros(NBLK, np.int64)
    blk_wins = [[] for _ in range(NBLK)]
    for qq in range(NQ):
        b0 = int(col_off[qq]) // WIN
        nb = int(collen[qq]) // WIN
        for w in range(NW):
            if maxcnt[qq, w] == 0:
                continue
            lo = int(wstart_in_col[qq, w])
            hi = int(seg_end_in_col[qq, w])
            for b in range(b0 + lo // WIN, b0 + (hi - 1) // WIN + 1):
                blk_wins[b].append(w)
        for b in range(b0, b0 + nb):
            ws = blk_wins[b]
            assert len(ws) <= 2, "block spans >2 windows"
            if len(ws) == 2:
                assert ws[1] == ws[0] + 1
            wmin_blk[b] = ws[0] if ws else 0

    rel = (o_dst - (o_core * SH + wmin_blk[blkid] * WIN)).astype(np.float32)
    assert rel.min() >= 0 and rel.max() <= 255.0

    per_core = []
    for c in range(NCORES):
        m = o_core == c
        idxq = np.zeros(TOT, np.int16)
        rels = np.full(TOT, -1.0, np.float32)
        p_c = pos[m]
        idxq[p_c] = o_tab[m].astype(np.int16)
        rels[p_c] = rel[m]
        idx16 = np.tile(np.ascontiguousarray(idxq.reshape(TOT // 16, 16).T), (8, 1))
        relm = np.ascontiguousarray(
            rels.reshape(NBLK, WIN).T.astype(np.int16))          # [128, NBLK]
        dinv_own = np.ascontiguousarray(
            dinv_np[c * SH:(c + 1) * SH].reshape(NW, WIN).T)     # [128, NW]
        per_core.append(dict(idx16=idx16, reldst=relm, dinv=dinv_own))

    # calls: per (g, q) a block range of column q. A block crossing the
    # group boundary is owned by the EARLIER call; its next-group matmuls
    # are deferred to that group ("pending"). Entry = (bcol, w, bank, stop).
    group_end_in_col = seg_end_in_col[
        :, [min((g + 1) * WG, NW) - 1 for g in range(NG)]]    # [NQ, NG]
    calls = []
    pend_of_group = [[] for _ in range(NG + 1)]   # (call_idx, bcol, w, bank)
    entries_of_group = [[] for _ in range(NG)]    # refs to mutable lists
    for gg in range(NG):
        for qq in range(NQ):
            b_col0 = int(col_off[qq]) // WIN
            own_lo = b_col0 if gg == 0 else calls[(gg - 1) * NQ + qq][4]
            pend = int(group_end_in_col[qq, gg])
            if gg == NG - 1:
                own_hi = b_col0 + int(collen[qq]) // WIN
            else:
                own_hi = b_col0 + (pend + WIN - 1) // WIN
            ci = len(calls)
            mms = []
            for b in range(own_lo, own_hi):
                for w in blk_wins[b]:
                    bank = w - blk_wins[b][0]
                    ent = [b, w, bank, False]
                    mms.append(ent)
                    g_of_w = w // WG
                    if g_of_w == gg:
                        entries_of_group[gg].append(ent)
                    else:
                        assert g_of_w == gg + 1, (g_of_w, gg)
                        pend_of_group[g_of_w].append((ci, ent))
            calls.append((gg, qq, own_lo * WIN, (own_hi - own_lo) * WIN,
                          own_hi, mms))
    # stop flag: per window, the LAST matmul in PE emission order. Emission
    # per group: pending entries (from the previous group's calls, in call
    # order) then this group's own-call entries in call order.
    for gg in range(NG):
        seq = [e for (_ci, e) in pend_of_group[gg]] + entries_of_group[gg]
        last = {}
        for ent in seq:
            last[ent[1]] = ent
        for ent in last.values():
            ent[3] = True
    sched = dict(calls=calls, pend_of_group=pend_of_group,
                 NBLK=NBLK, TOT=TOT)
    return sched, per_core, dinv_np, node_of_slot


def _build(sched, fast_epilogue=None):
    from concourse import bass, bacc, tile, mybir

    nc = bacc.Bacc("TRN2", target_bir_lowering=False, debug=False,
                   enable_asserts=True, num_devices=NCORES)

    xq_d = [nc.dram_tensor(f"xq{k}", [QTAB, KIN], mybir.dt.bfloat16,
                           kind="ExternalInput") for k in range(NQ)]
    w_d = nc.dram_tensor("w_bf", [KIN, H], mybir.dt.bfloat16, kind="ExternalInput")
    b_d = nc.dram_tensor("b_vec", [H], mybir.dt.float32, kind="ExternalInput")
    a_d = nc.dram_tensor("a_vec", [H], mybir.dt.float32, kind="ExternalInput")
    dinv_d = nc.dram_tensor("dinv_own", [128, NW], mybir.dt.float32, kind="ExternalInput")
    xown_d = nc.dram_tensor("xown", [128, NW * KIN], mybir.dt.float8e3, kind="ExternalInput")
    idx_d = nc.dram_tensor("idx16", [128, sched["TOT"] // 16], mybir.dt.int16, kind="ExternalInput")
    rel_d = nc.dram_tensor("reldst", [128, sched["NBLK"]], mybir.dt.int16, kind="ExternalInput")

    # output in partition-major layout: out[d, w*H + f] = result[w*128 + d, f]
    out_d = nc.dram_tensor("out_pm", [128, NW * H], mybir.dt.bfloat16, kind="ExternalOutput")

    calls = sched["calls"]
    pend_of_group = sched["pend_of_group"]
    max_call_blk = max(cb[3] // WIN for cb in calls)

    TOT16 = sched["TOT"] // 16
    NBLK = sched["NBLK"]

    with tile.TileContext(nc) as tc:
        with tc.tile_pool(name="consts", bufs=1) as cp, tc.tile_pool(name="sb", bufs=3) as sb:
            # all idx streams + rel columns resident in SBUF, loaded up front
            # in a few wide DMAs (>=512B per partition line avoids the 2x
            # small-descriptor penalty the per-call loads were paying). The
            # first chunk covers the first gather so it can start immediately.
            idx_all = cp.tile([128, TOT16], mybir.dt.int16, tag="idx_all")
            c0 = calls[0][3] // 16
            nc.sync.dma_start(idx_all[:, 0:c0], idx_d[:, 0:c0])
            c1 = min(TOT16, max(c0 + 256, TOT16 // 4))
            nc.sync.dma_start(idx_all[:, c0:c1], idx_d[:, c0:c1])
            rd_i16 = cp.tile([128, NBLK], mybir.dt.int16, tag="rd_i16")
            nc.sync.dma_start(rd_i16[:], rel_d[:, :])
            rd_all = cp.tile([128, NBLK], mybir.dt.float32, tag="rd_all")
            nc.vector.tensor_copy(rd_all[:], rd_i16[:])
            nc.sync.dma_start(idx_all[:, c1:TOT16], idx_d[:, c1:TOT16])

            # ---------------- constants ----------------
            iota_bank = []
            for j in range(2):
                it = cp.tile([128, 128], mybir.dt.int32, tag=f"it{j}")
                nc.gpsimd.iota(it[:], pattern=[[1, 128]], base=j * 128,
                               channel_multiplier=0)
                ib = cp.tile([128, 128], mybir.dt.bfloat16, tag=f"ib{j}")
                nc.vector.tensor_copy(ib[:], it[:])
                iota_bank.append(ib)

            # identity (bf16): I[p, d] = (d == p), for self-loop psum loads
            pidx = cp.tile([128, 1], mybir.dt.int32, tag="pidx")
            nc.gpsimd.iota(pidx[:], pattern=[[0, 1]], base=0, channel_multiplier=1)
            pidxf = cp.tile([128, 1], mybir.dt.float32, tag="pidxf")
            nc.vector.tensor_copy(pidxf[:], pidx[:])
            ident = cp.tile([128, 128], mybir.dt.bfloat16, tag="ident")
            nc.vector.tensor_scalar(
                out=ident[:], in0=iota_bank[0][:],
                scalar1=pidxf[:], scalar2=None,
                op0=mybir.AluOpType.is_equal)
            ident8 = cp.tile([128, 128], mybir.dt.float8e3, tag="ident8")
            nc.vector.tensor_copy(ident8[:], ident[:])

            w0 = cp.tile([128, H], mybir.dt.bfloat16)
            w1 = cp.tile([128, H], mybir.dt.bfloat16)
            nc.sync.dma_start(w0[:], w_d[0:128, :])
            nc.sync.dma_start(w1[:], w_d[128:256, :])

            dinv_sb = cp.tile([128, NW], mybir.dt.float32)
            nc.sync.dma_start(dinv_sb[:], dinv_d[:, :])

            ones1 = cp.tile([1, H], mybir.dt.float32)
            nc.vector.memset(ones1[:], 1.0)
            bvec = cp.tile([1, H], mybir.dt.float32)
            nc.sync.dma_start(bvec[:], b_d[None, :])
            avec = cp.tile([1, H], mybir.dt.float32)
            nc.sync.dma_start(avec[:], a_d[None, :])

            b128 = cp.tile([128, H], mybir.dt.float32)
            a128 = cp.tile([128, H], mybir.dt.float32)

            with tc.tile_pool(name="psum_o", bufs=2, space="PSUM") as ppo:
                if not fast_epilogue:
                    bc_ps = ppo.tile([128, H], mybir.dt.float32, space="PSUM", tag="op", bufs=2)
                    nc.tensor.matmul(out=bc_ps[:], lhsT=ones1[:], rhs=bvec[:], start=True, stop=True)
                    nc.vector.tensor_copy(b128[:], bc_ps[:])
                    ac_ps = ppo.tile([128, H], mybir.dt.float32, space="PSUM", tag="op", bufs=2)
                    nc.tensor.matmul(out=ac_ps[:], lhsT=ones1[:], rhs=avec[:], start=True, stop=True)
                    nc.vector.tensor_copy(a128[:], ac_ps[:])

                with tc.tile_pool(name="psum_agg", bufs=4, space="PSUM") as ppa:
                    # own-row (self-loop) x' slices, loaded per group one
                    # group ahead
                    xg_tiles = {}

                    def load_xg(gg):
                        wlo = gg * WG
                        whi = min(wlo + WG, NW)
                        xt = sb.tile([128, WG, KIN], mybir.dt.float8e3,
                                     tag="xg", bufs=3)
                        nc.sync.dma_start(
                            xt[:, 0:whi - wlo, :],
                            xown_d[:, wlo * KIN:whi * KIN].rearrange(
                                "p (w k) -> p w k", w=whi - wlo))
                        xg_tiles[gg] = xt

                    load_xg(0)

                    # per-call resources kept for pending (next-group) mms
                    call_gt = {}
                    call_b0 = {}

                    def build_s(bcol, bank):
                        s_t = sb.tile([128, 128], mybir.dt.bfloat16,
                                      tag="s_t", bufs=20)
                        nc.vector.tensor_scalar(
                            out=s_t[:], in0=iota_bank[bank][:],
                            scalar1=rd_all[:, bcol:bcol + 1], scalar2=None,
                            op0=mybir.AluOpType.is_equal)
                        return s_t

                    def emit_mm(agg, g_t, b_local, s_t, w, stop):
                        for c in range(2):
                            nc.tensor.matmul(
                                out=agg[(w, c)],
                                lhsT=g_t[:, b_local, c * 128:(c + 1) * 128],
                                rhs=s_t[:],
                                start=False, stop=stop)

                    def emit_window(w, wlo, whi, agg_tiles, o_g, asb_box,
                                    split_copy=False):
                        if w % 2 == 0:
                            # one wide copy per bank-pair; in the drain tail
                            # split it across Act and DVE so both halves run
                            # in parallel
                            a_sb = sb.tile([128, 4, 128], mybir.dt.bfloat16,
                                           tag="a_sb", bufs=4)
                            at = agg_tiles[(w - wlo) // 2]
                            if split_copy:
                                nc.scalar.activation(
                                    a_sb[:, 0:2, :], at[:, 0:2, :],
                                    mybir.ActivationFunctionType.Copy)
                                nc.vector.tensor_copy(a_sb[:, 2:4, :],
                                                      at[:, 2:4, :])
                            else:
                                nc.scalar.activation(
                                    a_sb[:], at[:],
                                    mybir.ActivationFunctionType.Copy)
                            asb_box[0] = a_sb
                        a_sb = asb_box[0]
                        op = ppo.tile([128, H], mybir.dt.float32, space="PSUM",
                                      tag="op", bufs=2)
                        nc.tensor.matmul(out=op[:], lhsT=a_sb[:, (w % 2) * 2, :],
                                         rhs=w0[:], start=True, stop=False)
                        nc.tensor.matmul(out=op[:], lhsT=a_sb[:, (w % 2) * 2 + 1, :],
                                         rhs=w1[:], start=False, stop=True)

                        og = o_g[:, (w - wlo) * H:(w - wlo + 1) * H]
                        if fast_epilogue:
                            # b == 0, uniform alpha: out = prelu(dinv*psum)
                            al = float(fast_epilogue["alpha"])
                            nc.scalar.activation(og, op[:],
                                                 mybir.ActivationFunctionType.Prelu,
                                                 scale=dinv_sb[:, w:w + 1],
                                                 alpha=al)
                        else:
                            u = sb.tile([128, H], mybir.dt.float32, tag="u", bufs=4)
                            nc.scalar.activation(u[:], op[:],
                                                 mybir.ActivationFunctionType.Copy,
                                                 scale=dinv_sb[:, w:w + 1])
                            u2 = sb.tile([128, H], mybir.dt.float32, tag="u2", bufs=4)
                            nc.vector.tensor_tensor(out=u2[:], in0=u[:], in1=b128[:],
                                                    op=mybir.AluOpType.add)
                            r2 = sb.tile([128, H], mybir.dt.float32, tag="r2", bufs=3)
                            nc.scalar.activation(r2[:], u2[:],
                                                 mybir.ActivationFunctionType.Relu,
                                                 scale=-1.0)
                            m = sb.tile([128, H], mybir.dt.float32, tag="m", bufs=3)
                            nc.gpsimd.tensor_tensor(out=m[:], in0=r2[:], in1=a128[:],
                                                    op=mybir.AluOpType.mult)
                            r1 = sb.tile([128, H], mybir.dt.float32, tag="r1", bufs=3)
                            nc.scalar.activation(r1[:], u2[:],
                                                 mybir.ActivationFunctionType.Relu)
                            nc.vector.tensor_tensor(out=og,
                                                    in0=r1[:], in1=m[:],
                                                    op=mybir.AluOpType.subtract)

                    for gg in range(NG):
                        wlo = gg * WG
                        whi = min(wlo + WG, NW)
                        nwin = whi - wlo
                        last_group = gg == NG - 1
                        xg = xg_tiles.pop(gg)
                        if gg + 1 < NG:
                            load_xg(gg + 1)
                        agg = {}
                        agg_tiles = {}
                        for w in range(wlo, whi):
                            if w % 2 == 0:
                                at = ppa.tile([128, 4, 128], mybir.dt.float32, space="PSUM",
                                              tag="agg", name=f"aggb{w // 2}", bufs=4)
                                agg_tiles[(w - wlo) // 2] = at
                            for c in range(2):
                                agg[(w, c)] = at[:, (w % 2) * 2 + c, :]
                                # self-loop row: aggT_c[k, d] = x'own[d, k]^T.
                                # start=True clears has_written for the WHOLE
                                # bank: only the first chain per bank sets it.
                                nc.tensor.matmul(
                                    out=agg[(w, c)],
                                    lhsT=xg[:, w - wlo, c * 128:(c + 1) * 128],
                                    rhs=ident8[:],
                                    start=(w % 2 == 0 and c == 0), stop=False)

                        # last group: emit each pair's projection + output as
                        # soon as both windows' stop matmuls are in
                        o_g = sb.tile([128, WG * H], mybir.dt.bfloat16, tag="o_g", bufs=2)
                        stops_done = set()
                        pairs_done = set()
                        asb_box = [None]

                        def on_stop(w):
                            stops_done.add(w)
                            pb = (w - wlo) // 2
                            wins = {wlo + 2 * pb}
                            if wlo + 2 * pb + 1 < whi:
                                wins.add(wlo + 2 * pb + 1)
                            if pb not in pairs_done and wins <= stops_done:
                                pairs_done.add(pb)
                                for wv in sorted(wins):
                                    emit_window(wv, wlo, whi, agg_tiles, o_g,
                                                asb_box, split_copy=True)
                                # eager per-pair output, alternating DMA
                                # queues so desc-gens overlap the Act chain
                                eng = nc.sync if pb % 2 == 0 else nc.scalar
                                eng.dma_start(
                                    out_d[:, (wlo + 2 * pb) * H:
                                          (wlo + 2 * pb + len(wins)) * H],
                                    o_g[:, 2 * pb * H:(2 * pb + len(wins)) * H])

                        # pending matmuls from group-boundary blocks gathered
                        # by the previous group's calls
                        for (pci, ent) in pend_of_group[gg]:
                            bcol, w, bank, stop = ent
                            s_t = build_s(bcol, bank)
                            p_halves, p_bmid = call_gt[pci]
                            ht, hlo = p_halves[0 if bcol < p_bmid else 1]
                            emit_mm(agg, ht, bcol - call_b0[pci] - hlo, s_t, w, stop)
                            if stop and last_group:
                                on_stop(w)

                        for ci, (g_c, qq, off_idx, nidx, _ohi, mms) in enumerate(calls):
                            if g_c != gg:
                                continue
                            # each call's gather is split in half, each half
                            # into its own (smaller) tile: more pool buffers
                            # fit in SBUF, so transfers are not starved
                            # waiting on matmul drain, and desc-gen overlaps.
                            nb_c = nidx // 128
                            bmid = (nb_c + 1) // 2
                            nsplit0 = 4 if ci == 0 else 1
                            nsplit1 = 2 if ci == len(calls) - 1 else 1
                            halves = []
                            for (hlo, hhi, nsp) in ((0, bmid, nsplit0),
                                                    (bmid, nb_c, nsplit1)):
                                ht = sb.tile([128, (max_call_blk + 1) // 2, KIN],
                                             mybir.dt.bfloat16, tag="g_t", bufs=12)
                                halves.append((ht, hlo))
                                if hhi == hlo:
                                    continue
                                cuts = [hlo + round(j * (hhi - hlo) / nsp)
                                        for j in range(nsp + 1)]
                                for j in range(nsp):
                                    blo, bhi = cuts[j], cuts[j + 1]
                                    if bhi == blo:
                                        continue
                                    nsub = (bhi - blo) * 128
                                    nc.gpsimd.dma_gather(
                                        ht[:, blo - hlo:bhi - hlo, :], xq_d[qq][:, :],
                                        idx_all[:, off_idx // 16 + blo * 8:
                                                off_idx // 16 + blo * 8 + nsub // 16],
                                        nsub, nsub, KIN,
                                        single_packet=False)
                            b0 = off_idx // WIN
                            call_gt[ci] = (halves, b0 + bmid)
                            call_b0[ci] = b0
                            for ent in mms:
                                bcol, w, bank, stop = ent
                                if w // WG != gg:
                                    continue       # deferred to next group
                                s_t = build_s(bcol, bank)
                                ht, hlo = halves[0 if bcol - b0 < bmid else 1]
                                emit_mm(agg, ht, bcol - b0 - hlo, s_t, w, stop)
                                if stop and last_group:
                                    on_stop(w)

                        if not last_group:
                            # projection + epilogue (eager per-pair with
                            # per-pair output DMA for the last group)
                            for w in range(wlo, whi):
                                emit_window(w, wlo, whi, agg_tiles, o_g, asb_box)
                            nc.scalar.dma_start(
                                out_d[:, wlo * H:whi * H],
                                o_g[:, 0:nwin * H])

    nc.compile()
    return nc


_LAST = {}


def kernel(x, edge_index, W, b, alpha):
    from concourse.bass_utils import run_bass_kernel_spmd

    x = np.asarray(x, dtype=np.float32)
    W = np.asarray(W, dtype=np.float32)
    b = np.asarray(b, dtype=np.float32)
    alpha = np.asarray(alpha, dtype=np.float32)

    sched, per_core, dinv_np, node_of_slot = _preprocess(edge_index)
    fast = None
    if np.all(b == 0.0) and np.all(alpha == alpha.flat[0]) and 0.0 <= alpha.flat[0] <= 1.0:
        fast = {"alpha": float(alpha.flat[0])}
    nc = _build(sched, fast_epilogue=fast)
    _LAST["nc"] = nc
    _LAST["sched"] = sched

    # x' = dinv * x, in NODE order (gather src tables); quarter tables
    # shared by all cores.
    deg = (np.bincount(np.asarray(edge_index[1]).astype(np.int64),
                       minlength=N) + 1).astype(np.float32)
    dinv_node = (1.0 / np.sqrt(deg)).astype(np.float32)
    x_pad = np.zeros((NP, KIN), np.float32)
    x_pad[:N] = dinv_node[:, None] * x
    x_bf = x_pad.astype(bf16)
    xq = [np.ascontiguousarray(x_bf[k * QTAB:(k + 1) * QTAB]) for k in range(NQ)]

    # x' rows in SLOT order for self-loops (pad slots -> zero row); fp8
    # e3m4 is plenty: the self-loop is ~1/sqrt(deg) of the output, so the
    # ~1.4% element error contributes ~0.3% globally
    f8 = ml_dtypes.float8_e3m4
    x_slot = np.zeros((NP, KIN), f8)
    real = node_of_slot >= 0
    x_slot[real] = x_pad[node_of_slot[real]].astype(f8)

    w_bf = W.astype(bf16)

    in_maps = []
    for c in range(NCORES):
        xown = np.ascontiguousarray(
            x_slot[c * SH:(c + 1) * SH].reshape(NW, 128, KIN)
            .transpose(1, 0, 2).reshape(128, NW * KIN))
        im = {
            "w_bf": w_bf, "b_vec": b, "a_vec": alpha,
            "dinv_own": per_core[c]["dinv"],
            "xown": xown,
            "idx16": per_core[c]["idx16"],
            "reldst": per_core[c]["reldst"],
        }
        for k in range(NQ):
            im[f"xq{k}"] = xq[k]
        in_maps.append(im)

    res = run_bass_kernel_spmd(nc, in_maps, core_ids=list(range(NCORES)))
    # out_pm[d, w*H+f] -> slot w*128+d; slot -> node via node_of_slot
    outs = []
    for c in range(NCORES):
        o = res.results[c]["out_pm"].astype(np.float32).reshape(128, NW, H).transpose(1, 0, 2)
        outs.append(o.reshape(SH, H))
    out_slots = np.concatenate(outs, axis=0)
    out = np.empty((N, H), np.float32)
    out[node_of_slot[real]] = out_slots[real]
    return out


# revision 84
# speedup vs baseline: 1.0000x; 1.0000x over previous
"""GCN layer (gather -> aggregate -> @W -> bias -> PReLU) on 8 trn2 cores.

Strategy (v4: aggregate pre-projection features, project per dst window):
  - out[d] = prelu(dinv_d * (agg[d] @ W) + b), agg[d] = x'[d] + sum_{e: dst=d}
    x'[src_e], where x' = dinv[:,None] * x is precomputed on host (folds the
    src-side norm; self-loop term dinv_d^2 * x[d] @ W comes from the x'[d]
    row). Linearity lets us aggregate 256-dim inputs first and apply W once
    per dst, so no hs table is materialized on device.
  - 100000 nodes -> 102400 slots = 8 cores x 100 windows x 128; nodes are
    dealt to (core, window) by a per-window greedy that equalizes
    per-(window, src-quarter) in-edge counts across cores (the shared SPMD
    schedule pads every segment to the max count over cores, so balance is
    throughput). x' staged as 4 quarter tables [25600, 256] bf16 (identical
    on every core; gather idx fit int16); gather elem = 512B (mult-1 DMA).
  - Schedule layout is quarter-major: per q, the 13 window-groups'
    per-window max-count segments are concatenated with NO padding between
    groups (only each q column is padded to 128). 128-edge blocks may
    straddle a group boundary; their next-group matmuls are deferred
    ("pending") until that group's PSUM tiles open.
  - All gather idx streams and per-block rel columns are SBUF-resident,
    loaded in a few wide DMAs (>=512B per partition line dodges the 2x
    small-descriptor penalty). Each call's gather is split into two
    half-sized tiles for deep pipelining. Per block a one-hot S[e, d] =
    (iota_bank == rel[e]) is built on DVE (rel is dst offset from the
    block's first window, 0..255, bf16-exact); PE
    accumulates aggT_c[k, d] += G_c^T @ S per overlapped window (c = k-chunk
    of 2), G = dma_gather'ed x' rows. Self-loop rows enter each window's aggT
    via identity matmuls from SBUF-resident x'own slices (start=True only on
    the first chain per PSUM bank: start clears the whole bank's has_written
    bits).
  - Per window-pair: aggT bank copied wide to bf16 SBUF, per window projected
    out_psum[d, f] = sum_c aggT_c^T @ W_c. Epilogue: out = prelu(dinv_dst *
    psum + b) as max(u, alpha*u) when b == 0 and alpha uniform in [0,1]
    (true here); bf16 output written partition-major [128, NW*H], upcast +
    unpermuted on host.
"""
import sys
sys.path.insert(0, '/opt/trn_rl_repo')

import numpy as np
import ml_dtypes

N = 100000
NCORES = 8
SH = 12800                 # dst slots per core
NP = NCORES * SH           # 102400 slots
H = 128                    # output features
KIN = 256                  # input features
WIN = 128                  # dst window size
NW = SH // WIN             # 100 windows per core
WG = 8                     # windows per PSUM group
NG = (NW + WG - 1) // WG   # 13 groups (last has 4)
NQ = 4                     # source quarters (int16 gather idx: 25600 < 32768)
QTAB = NP // NQ            # 25600 rows per quarter table
bf16 = ml_dtypes.bfloat16


def _balance_nodes(e_dst, e_src):
    """Assign nodes to (core, window, slot) so that per-(window, quarter)
    in-edge counts are nearly equal across the 8 cores. Greedy on the true
    objective (growth of sum_q max_c load) + a bounded swap pass.
    Returns node_of_slot [NP] (-1 for pad slots)."""
    q = e_src // QTAB
    qd = np.bincount(e_dst * NQ + q, minlength=N * NQ).reshape(N, NQ)
    tot = qd.sum(1)
    order = np.argsort(-tot, kind='stable')

    node_of_slot = np.full(NP, -1, np.int64)
    per_w = N // NW
    assert per_w * NW == N and per_w <= NCORES * WIN
    for w in range(NW):
        chunk = order[w * per_w:(w + 1) * per_w]
        L = np.zeros((NCORES, NQ), np.int64)
        members = [[] for _ in range(NCORES)]
        for n in chunk:
            d = qd[n]
            Ld = L + d[None, :]
            mx = L.max(axis=0)
            cost = np.maximum(Ld - mx[None, :], 0).sum(1)
            full = np.array([len(m) >= WIN for m in members])
            c = int(np.argmin(cost + L.sum(1) * 1e-6 + full * (1 << 40)))
            members[c].append(n)
            L[c] += d
        # bounded swap pass among the lightest tail nodes
        for _ in range(6):
            improved = False
            for _try in range(400):
                mx = L.max(axis=0)
                obj = mx.sum()
                cs = int(np.argmax(L[:, np.argmax(mx - L.min(axis=0))]))
                best = None
                for n_i in range(max(0, len(members[cs]) - 6), len(members[cs])):
                    n = members[cs][n_i]
                    for ct in range(NCORES):
                        if ct == cs:
                            continue
                        for m_i in range(max(0, len(members[ct]) - 6), len(members[ct])):
                            m = members[ct][m_i]
                            L2 = L.copy()
                            L2[cs] += qd[m] - qd[n]
                            L2[ct] += qd[n] - qd[m]
                            no = L2.max(axis=0).sum()
                            if no < obj - 0.5:
                                best = (n_i, cs, m_i, ct)
                                obj = no
                if best is None:
                    break
                n_i, cs2, m_i, ct = best
                n, m = members[cs2][n_i], members[ct][m_i]
                members[cs2][n_i] = m
                members[ct][m_i] = n
                L[cs2] += qd[m] - qd[n]
                L[ct] += qd[n] - qd[m]
                improved = True
            if not improved:
                break
        for c in range(NCORES):
            for i, n in enumerate(members[c]):
                node_of_slot[c * SH + w * WIN + i] = n
    return node_of_slot


def _preprocess(edge_index):
    e_src = np.asarray(edge_index[0]).astype(np.int64)
    e_dst = np.asarray(edge_index[1]).astype(np.int64)

    deg = (np.bincount(e_dst, minlength=N) + 1).astype(np.float32)
    dinv = (1.0 / np.sqrt(deg)).astype(np.float32)

    node_of_slot = _balance_nodes(e_dst, e_src)
    real = node_of_slot >= 0
    slot_of_node = np.empty(N, np.int64)
    slot_of_node[node_of_slot[real]] = np.nonzero(real)[0]

    dinv_np = np.ones(NP, np.float32)           # indexed by SLOT
    dinv_np[real] = dinv[node_of_slot[real]]

    # real edges only; self-loops are applied from SBUF
    src = e_src
    dst = slot_of_node[e_dst]                   # dst position = slot
    E = src.shape[0]

    core = dst // SH
    w_all = (dst % SH) // WIN                # 0..NW-1
    g = w_all // WG
    q = src // QTAB                          # source quarter
    tab_row = src % QTAB                     # row within quarter table

    # sort by (core, q, window)  [quarter-major layout]
    key = (core * NQ + q) * NW + w_all
    nbins_pc = NQ * NW
    order = np.argsort(key, kind='stable')
    o_tab = tab_row[order]
    o_dst = dst[order]
    o_key = key[order]
    o_core = core[order]

    cnt_all = np.bincount(key, minlength=NCORES * nbins_pc)
    bin_start = np.concatenate([[0], np.cumsum(cnt_all)])[:-1]
    rank = np.arange(E, dtype=np.int64) - bin_start[o_key]   # within (c,q,w)

    # every window must have at least one real edge on SOME core, so the
    # shared schedule has a stop-flag anchor for each PSUM region
    wcnt = cnt_all.reshape(NCORES, NQ, NW).sum(axis=(0, 1))
    assert wcnt.min() >= 1

    # shared schedule: per (q,w) segment length = max count over cores; per
    # q column the segments are concatenated (window-major) and only the
    # column end is padded to a 128 multiple.
    maxcnt = cnt_all.reshape(NCORES, NQ, NW).max(axis=0)      # [NQ, NW]
    seg_end_in_col = np.cumsum(maxcnt, axis=1)                # [NQ, NW]
    wstart_in_col = seg_end_in_col - maxcnt
    collen = ((seg_end_in_col[:, -1] + WIN - 1) // WIN) * WIN  # [NQ]
    col_off = np.concatenate([[0], np.cumsum(collen)])
    TOT = int(col_off[-1])
    NBLK = TOT // WIN

    qw_in_core = o_key % nbins_pc
    q_in_core = qw_in_core // NW
    pos = (col_off[q_in_core] + wstart_in_col.reshape(-1)[qw_in_core] + rank)
    blkid = pos // WIN                                        # global block

    # per-block overlapped windows from the shared segment layout
    wmin_blk = np.zeros(NBLK, np.int64)
    blk_wins = [[] for _ in range(NBLK)]
    for qq in range(NQ):
        b0 = int(col_off[qq]) // WIN
        nb = int(collen[qq]) // WIN
        for w in range(NW):
            if maxcnt[qq, w] == 0:
                continue
            lo = int(wstart_in_col[qq, w])
            hi = int(seg_end_in_col[qq, w])
            for b in range(b0 + lo // WIN, b0 + (hi - 1) // WIN + 1):
                blk_wins[b].append(w)
        for b in range(b0, b0 + nb):
            ws = blk_wins[b]
            assert len(ws) <= 2, "block spans >2 windows"
            if len(ws) == 2:
                assert ws[1] == ws[0] + 1
            wmin_blk[b] = ws[0] if ws else 0

    rel = (o_dst - (o_core * SH + wmin_blk[blkid] * WIN)).astype(np.float32)
    assert rel.min() >= 0 and rel.max() <= 255.0

    per_core = []
    for c in range(NCORES):
        m = o_core == c
        idxq = np.zeros(TOT, np.int16)
        rels = np.full(TOT, -1.0, np.float32)
        p_c = pos[m]
        idxq[p_c] = o_tab[m].astype(np.int16)
        rels[p_c] = rel[m]
        idx16 = np.tile(np.ascontiguousarray(idxq.reshape(TOT // 16, 16).T), (8, 1))
        relm = np.ascontiguousarray(
            rels.reshape(NBLK, WIN).T.astype(np.int16))          # [128, NBLK]
        dinv_own = np.ascontiguousarray(
            dinv_np[c * SH:(c + 1) * SH].reshape(NW, WIN).T)     # [128, NW]
        per_core.append(dict(idx16=idx16, reldst=relm, dinv=dinv_own))

    # calls: per (g, q) a block range of column q. A block crossing the
    # group boundary is owned by the EARLIER call; its next-group matmuls
    # are deferred to that group ("pending"). Entry = (bcol, w, bank, stop).
    group_end_in_col = seg_end_in_col[
        :, [min((g + 1) * WG, NW) - 1 for g in range(NG)]]    # [NQ, NG]
    calls = []
    pend_of_group = [[] for _ in range(NG + 1)]   # (call_idx, bcol, w, bank)
    entries_of_group = [[] for _ in range(NG)]    # refs to mutable lists
    for gg in range(NG):
        for qq in range(NQ):
            b_col0 = int(col_off[qq]) // WIN
            own_lo = b_col0 if gg == 0 else calls[(gg - 1) * NQ + qq][4]
            pend = int(group_end_in_col[qq, gg])
            if gg == NG - 1:
                own_hi = b_col0 + int(collen[qq]) // WIN
            else:
                own_hi = b_col0 + (pend + WIN - 1) // WIN
            ci = len(calls)
            mms = []
            for b in range(own_lo, own_hi):
                for w in blk_wins[b]:
                    bank = w - blk_wins[b][0]
                    ent = [b, w, bank, False]
                    mms.append(ent)
                    g_of_w = w // WG
                    if g_of_w == gg:
                        entries_of_group[gg].append(ent)
                    else:
                        assert g_of_w == gg + 1, (g_of_w, gg)
                        pend_of_group[g_of_w].append((ci, ent))
            calls.append((gg, qq, own_lo * WIN, (own_hi - own_lo) * WIN,
                          own_hi, mms))
    # stop flag: per window, the LAST matmul in PE emission order. Emission
    # per group: pending entries (from the previous group's calls, in call
    # order) then this group's own-call entries in call order.
    for gg in range(NG):
        seq = [e for (_ci, e) in pend_of_group[gg]] + entries_of_group[gg]
        last = {}
        for ent in seq:
            last[ent[1]] = ent
        for ent in last.values():
            ent[3] = True
    sched = dict(calls=calls, pend_of_group=pend_of_group,
                 NBLK=NBLK, TOT=TOT)
    return sched, per_core, dinv_np, node_of_slot


def _build(sched, fast_epilogue=None):
    from concourse import bass, bacc, tile, mybir

    nc = bacc.Bacc("TRN2", target_bir_lowering=False, debug=False,
                   enable_asserts=True, num_devices=NCORES)

    xq_d = [nc.dram_tensor(f"xq{k}", [QTAB, KIN], mybir.dt.bfloat16,
                           kind="ExternalInput") for k in range(NQ)]
    w_d = nc.dram_tensor("w_bf", [KIN, H], mybir.dt.bfloat16, kind="ExternalInput")
    b_d = nc.dram_tensor("b_vec", [H], mybir.dt.float32, kind="ExternalInput")
    a_d = nc.dram_tensor("a_vec", [H], mybir.dt.float32, kind="ExternalInput")
    dinv_d = nc.dram_tensor("dinv_own", [128, NW], mybir.dt.float32, kind="ExternalInput")
    xown_d = nc.dram_tensor("xown", [128, NW * KIN], mybir.dt.float8e3, kind="ExternalInput")
    idx_d = nc.dram_tensor("idx16", [128, sched["TOT"] // 16], mybir.dt.int16, kind="ExternalInput")
    rel_d = nc.dram_tensor("reldst", [128, sched["NBLK"]], mybir.dt.int16, kind="ExternalInput")

    # output in partition-major layout: out[d, w*H + f] = result[w*128 + d, f]
    out_d = nc.dram_tensor("out_pm", [128, NW * H], mybir.dt.bfloat16, kind="ExternalOutput")

    calls = sched["calls"]
    pend_of_group = sched["pend_of_group"]
    max_call_blk = max(cb[3] // WIN for cb in calls)

    TOT16 = sched["TOT"] // 16
    NBLK = sched["NBLK"]

    with tile.TileContext(nc) as tc:
        with tc.tile_pool(name="consts", bufs=1) as cp, tc.tile_pool(name="sb", bufs=3) as sb:
            # all idx streams + rel columns resident in SBUF, loaded up front
            # in a few wide DMAs (>=512B per partition line avoids the 2x
            # small-descriptor penalty the per-call loads were paying). The
            # first chunk covers the first gather so it can start immediately.
            idx_all = cp.tile([128, TOT16], mybir.dt.int16, tag="idx_all")
            c0 = calls[0][3] // 16
            nc.sync.dma_start(idx_all[:, 0:c0], idx_d[:, 0:c0])
            c1 = min(TOT16, max(c0 + 256, TOT16 // 4))
            nc.sync.dma_start(idx_all[:, c0:c1], idx_d[:, c0:c1])
            rd_i16 = cp.tile([128, NBLK], mybir.dt.int16, tag="rd_i16")
            nc.sync.dma_start(rd_i16[:], rel_d[:, :])
            rd_all = cp.tile([128, NBLK], mybir.dt.float32, tag="rd_all")
            nc.vector.tensor_copy(rd_all[:], rd_i16[:])
            nc.sync.dma_start(idx_all[:, c1:TOT16], idx_d[:, c1:TOT16])

            # ---------------- constants ----------------
            iota_bank = []
            for j in range(2):
                it = cp.tile([128, 128], mybir.dt.int32, tag=f"it{j}")
                nc.gpsimd.iota(it[:], pattern=[[1, 128]], base=j * 128,
                               channel_multiplier=0)
                ib = cp.tile([128, 128], mybir.dt.bfloat16, tag=f"ib{j}")
                nc.vector.tensor_copy(ib[:], it[:])
                iota_bank.append(ib)

            # identity (bf16): I[p, d] = (d == p), for self-loop psum loads
            pidx = cp.tile([128, 1], mybir.dt.int32, tag="pidx")
            nc.gpsimd.iota(pidx[:], pattern=[[0, 1]], base=0, channel_multiplier=1)
            pidxf = cp.tile([128, 1], mybir.dt.float32, tag="pidxf")
            nc.vector.tensor_copy(pidxf[:], pidx[:])
            ident = cp.tile([128, 128], mybir.dt.bfloat16, tag="ident")
            nc.vector.tensor_scalar(
                out=ident[:], in0=iota_bank[0][:],
                scalar1=pidxf[:], scalar2=None,
                op0=mybir.AluOpType.is_equal)
            ident8 = cp.tile([128, 128], mybir.dt.float8e3, tag="ident8")
            nc.vector.tensor_copy(ident8[:], ident[:])

            w0 = cp.tile([128, H], mybir.dt.bfloat16)
            w1 = cp.tile([128, H], mybir.dt.bfloat16)
            nc.sync.dma_start(w0[:], w_d[0:128, :])
            nc.sync.dma_start(w1[:], w_d[128:256, :])

            dinv_sb = cp.tile([128, NW], mybir.dt.float32)
            nc.sync.dma_start(dinv_sb[:], dinv_d[:, :])

            ones1 = cp.tile([1, H], mybir.dt.float32)
            nc.vector.memset(ones1[:], 1.0)
            bvec = cp.tile([1, H], mybir.dt.float32)
            nc.sync.dma_start(bvec[:], b_d[None, :])
            avec = cp.tile([1, H], mybir.dt.float32)
            nc.sync.dma_start(avec[:], a_d[None, :])

            b128 = cp.tile([128, H], mybir.dt.float32)
            a128 = cp.tile([128, H], mybir.dt.float32)

            with tc.tile_pool(name="psum_o", bufs=2, space="PSUM") as ppo:
                if not fast_epilogue:
                    bc_ps = ppo.tile([128, H], mybir.dt.float32, space="PSUM", tag="op", bufs=2)
                    nc.tensor.matmul(out=bc_ps[:], lhsT=ones1[:], rhs=bvec[:], start=True, stop=True)
                    nc.vector.tensor_copy(b128[:], bc_ps[:])
                    ac_ps = ppo.tile([128, H], mybir.dt.float32, space="PSUM", tag="op", bufs=2)
                    nc.tensor.matmul(out=ac_ps[:], lhsT=ones1[:], rhs=avec[:], start=True, stop=True)
                    nc.vector.tensor_copy(a128[:], ac_ps[:])

                with tc.tile_pool(name="psum_agg", bufs=4, space="PSUM") as ppa:
                    # own-row (self-loop) x' slices, loaded per group one
                    # group ahead
                    xg_tiles = {}

                    def load_xg(gg):
                        wlo = gg * WG
                        whi = min(wlo + WG, NW)
                        xt = sb.tile([128, WG, KIN], mybir.dt.float8e3,
                                     tag="xg", bufs=3)
                        nc.sync.dma_start(
                            xt[:, 0:whi - wlo, :],
                            xown_d[:, wlo * KIN:whi * KIN].rearrange(
                                "p (w k) -> p w k", w=whi - wlo))
                        xg_tiles[gg] = xt

                    load_xg(0)

                    # per-call resources kept for pending (next-group) mms
                    call_gt = {}
                    call_b0 = {}

                    def build_s(bcol, bank):
                        s_t = sb.tile([128, 128], mybir.dt.bfloat16,
                                      tag="s_t", bufs=24)
                        nc.vector.tensor_scalar(
                            out=s_t[:], in0=iota_bank[bank][:],
                            scalar1=rd_all[:, bcol:bcol + 1], scalar2=None,
                            op0=mybir.AluOpType.is_equal)
                        return s_t

                    def emit_mm(agg, g_t, b_local, s_t, w, stop):
                        for c in range(2):
                            nc.tensor.matmul(
                                out=agg[(w, c)],
                                lhsT=g_t[:, b_local, c * 128:(c + 1) * 128],
                                rhs=s_t[:],
                                start=False, stop=stop)

                    def emit_window(w, wlo, whi, agg_tiles, o_g, asb_box,
                                    split_copy=False):
                        if w % 2 == 0:
                            # one wide copy per bank-pair; in the drain tail
                            # split it across Act and DVE so both halves run
                            # in parallel
                            a_sb = sb.tile([128, 4, 128], mybir.dt.bfloat16,
                                           tag="a_sb", bufs=4)
                            at = agg_tiles[(w - wlo) // 2]
                            if split_copy:
                                nc.scalar.activation(
                                    a_sb[:, 0:2, :], at[:, 0:2, :],
                                    mybir.ActivationFunctionType.Copy)
                                nc.vector.tensor_copy(a_sb[:, 2:4, :],
                                                      at[:, 2:4, :])
                            else:
                                nc.scalar.activation(
                                    a_sb[:], at[:],
                                    mybir.ActivationFunctionType.Copy)
                            asb_box[0] = a_sb
                        a_sb = asb_box[0]
                        op = ppo.tile([128, H], mybir.dt.float32, space="PSUM",
                                      tag="op", bufs=2)
                        nc.tensor.matmul(out=op[:], lhsT=a_sb[:, (w % 2) * 2, :],
                                         rhs=w0[:], start=True, stop=False)
                        nc.tensor.matmul(out=op[:], lhsT=a_sb[:, (w % 2) * 2 + 1, :],
                                         rhs=w1[:], start=False, stop=True)

                        og = o_g[:, (w - wlo) * H:(w - wlo + 1) * H]
                        if fast_epilogue:
                            # b == 0, uniform alpha: out = prelu(dinv*psum)
                            al = float(fast_epilogue["alpha"])
                            nc.scalar.activation(og, op[:],
                                                 mybir.ActivationFunctionType.Prelu,
                                                 scale=dinv_sb[:, w:w + 1],
                                                 alpha=al)
                        else:
                            u = sb.tile([128, H], mybir.dt.float32, tag="u", bufs=4)
                            nc.scalar.activation(u[:], op[:],
                                                 mybir.ActivationFunctionType.Copy,
                                                 scale=dinv_sb[:, w:w + 1])
                            u2 = sb.tile([128, H], mybir.dt.float32, tag="u2", bufs=4)
                            nc.vector.tensor_tensor(out=u2[:], in0=u[:], in1=b128[:],
                                                    op=mybir.AluOpType.add)
                            r2 = sb.tile([128, H], mybir.dt.float32, tag="r2", bufs=3)
                            nc.scalar.activation(r2[:], u2[:],
                                                 mybir.ActivationFunctionType.Relu,
                                                 scale=-1.0)
                            m = sb.tile([128, H], mybir.dt.float32, tag="m", bufs=3)
                            nc.gpsimd.tensor_tensor(out=m[:], in0=r2[:], in1=a128[:],
                                                    op=mybir.AluOpType.mult)
                            r1 = sb.tile([128, H], mybir.dt.float32, tag="r1", bufs=3)
                            nc.scalar.activation(r1[:], u2[:],
                                                 mybir.ActivationFunctionType.Relu)
                            nc.vector.tensor_tensor(out=og,
                                                    in0=r1[:], in1=m[:],
                                                    op=mybir.AluOpType.subtract)

                    for gg in range(NG):
                        wlo = gg * WG
                        whi = min(wlo + WG, NW)
                        nwin = whi - wlo
                        last_group = gg == NG - 1
                        xg = xg_tiles.pop(gg)
                        if gg + 1 < NG:
                            load_xg(gg + 1)
                        agg = {}
                        agg_tiles = {}
                        for w in range(wlo, whi):
                            if w % 2 == 0:
                                at = ppa.tile([128, 4, 128], mybir.dt.float32, space="PSUM",
                                              tag="agg", name=f"aggb{w // 2}", bufs=4)
                                agg_tiles[(w - wlo) // 2] = at
                            for c in range(2):
                                agg[(w, c)] = at[:, (w % 2) * 2 + c, :]
                                # self-loop row: aggT_c[k, d] = x'own[d, k]^T.
                                # start=True clears has_written for the WHOLE
                                # bank: only the first chain per bank sets it.
                                nc.tensor.matmul(
                                    out=agg[(w, c)],
                                    lhsT=xg[:, w - wlo, c * 128:(c + 1) * 128],
                                    rhs=ident8[:],
                                    start=(w % 2 == 0 and c == 0), stop=False)

                        # last group: emit each pair's projection + output as
                        # soon as both windows' stop matmuls are in
                        o_g = sb.tile([128, WG * H], mybir.dt.bfloat16, tag="o_g", bufs=2)
                        stops_done = set()
                        pairs_done = set()
                        asb_box = [None]

                        def on_stop(w):
                            stops_done.add(w)
                            pb = (w - wlo) // 2
                            wins = {wlo + 2 * pb}
                            if wlo + 2 * pb + 1 < whi:
                                wins.add(wlo + 2 * pb + 1)
                            if pb not in pairs_done and wins <= stops_done:
                                pairs_done.add(pb)
                                for wv in sorted(wins):
                                    emit_window(wv, wlo, whi, agg_tiles, o_g,
                                                asb_box, split_copy=True)
                                # eager per-pair output, alternating DMA
                                # queues so desc-gens overlap the Act chain
                                eng = nc.sync if pb % 2 == 0 else nc.scalar
                                eng.dma_start(
                                    out_d[:, (wlo + 2 * pb) * H:
                                          (wlo + 2 * pb + len(wins)) * H],
                                    o_g[:, 2 * pb * H:(2 * pb + len(wins)) * H])

                        # pending matmuls from group-boundary blocks gathered
                        # by the previous group's calls
                        for (pci, ent) in pend_of_group[gg]:
                            bcol, w, bank, stop = ent
                            s_t = build_s(bcol, bank)
                            p_halves, p_bmid = call_gt[pci]
                            ht, hlo = p_halves[0 if bcol < p_bmid else 1]
                            emit_mm(agg, ht, bcol - call_b0[pci] - hlo, s_t, w, stop)
                            if stop and last_group:
                                on_stop(w)

                        for ci, (g_c, qq, off_idx, nidx, _ohi, mms) in enumerate(calls):
                            if g_c != gg:
                                continue
                            # each call's gather is split in half, each half
                            # into its own (smaller) tile: more pool buffers
                            # fit in SBUF, so transfers are not starved
                            # waiting on matmul drain, and desc-gen overlaps.
                            nb_c = nidx // 128
                            bmid = (nb_c + 1) // 2
                            nsplit0 = 4 if ci == 0 else 1
                            nsplit1 = 2 if ci == len(calls) - 1 else 1
                            halves = []
                            for (hlo, hhi, nsp) in ((0, bmid, nsplit0),
                                                    (bmid, nb_c, nsplit1)):
                                ht = sb.tile([128, (max_call_blk + 1) // 2, KIN],
                                             mybir.dt.bfloat16, tag="g_t", bufs=12)
                                halves.append((ht, hlo))
                                if hhi == hlo:
                                    continue
                                cuts = [hlo + round(j * (hhi - hlo) / nsp)
                                        for j in range(nsp + 1)]
                                for j in range(nsp):
                                    blo, bhi = cuts[j], cuts[j + 1]
                                    if bhi == blo:
                                        continue
                                    nsub = (bhi - blo) * 128
                                    nc.gpsimd.dma_gather(
                                        ht[:, blo - hlo:bhi - hlo, :], xq_d[qq][:, :],
                                        idx_all[:, off_idx // 16 + blo * 8:
                                                off_idx // 16 + blo * 8 + nsub // 16],
                                        nsub, nsub, KIN,
                                        single_packet=False)
                            b0 = off_idx // WIN
                            call_gt[ci] = (halves, b0 + bmid)
                            call_b0[ci] = b0
                            for ent in mms:
                                bcol, w, bank, stop = ent
                                if w // WG != gg:
                                    continue       # deferred to next group
                                s_t = build_s(bcol, bank)
                                ht, hlo = halves[0 if bcol - b0 < bmid else 1]
                                emit_mm(agg, ht, bcol - b0 - hlo, s_t, w, stop)
                                if stop and last_group:
                                    on_stop(w)

                        if not last_group:
                            # projection + epilogue (eager per-pair with
                            # per-pair output DMA for the last group)
                            for w in range(wlo, whi):
                                emit_window(w, wlo, whi, agg_tiles, o_g, asb_box)
                            nc.scalar.dma_start(
                                out_d[:, wlo * H:whi * H],
                                o_g[:, 0:nwin * H])

    nc.compile()
    return nc


_LAST = {}


def kernel(x, edge_index, W, b, alpha):
    from concourse.bass_utils import run_bass_kernel_spmd

    x = np.asarray(x, dtype=np.float32)
    W = np.asarray(W, dtype=np.float32)
    b = np.asarray(b, dtype=np.float32)
    alpha = np.asarray(alpha, dtype=np.float32)

    sched, per_core, dinv_np, node_of_slot = _preprocess(edge_index)
    fast = None
    if np.all(b == 0.0) and np.all(alpha == alpha.flat[0]) and 0.0 <= alpha.flat[0] <= 1.0:
        fast = {"alpha": float(alpha.flat[0])}
    nc = _build(sched, fast_epilogue=fast)
    _LAST["nc"] = nc
    _LAST["sched"] = sched

    # x' = dinv * x, in NODE order (gather src tables); quarter tables
    # shared by all cores.
    deg = (np.bincount(np.asarray(edge_index[1]).astype(np.int64),
                       minlength=N) + 1).astype(np.float32)
    dinv_node = (1.0 / np.sqrt(deg)).astype(np.float32)
    x_pad = np.zeros((NP, KIN), np.float32)
    x_pad[:N] = dinv_node[:, None] * x
    x_bf = x_pad.astype(bf16)
    xq = [np.ascontiguousarray(x_bf[k * QTAB:(k + 1) * QTAB]) for k in range(NQ)]

    # x' rows in SLOT order for self-loops (pad slots -> zero row); fp8
    # e3m4 is plenty: the self-loop is ~1/sqrt(deg) of the output, so the
    # ~1.4% element error contributes ~0.3% globally
    f8 = ml_dtypes.float8_e3m4
    x_slot = np.zeros((NP, KIN), f8)
    real = node_of_slot >= 0
    x_slot[real] = x_pad[node_of_slot[real]].astype(f8)

    w_bf = W.astype(bf16)

    in_maps = []
    for c in range(NCORES):
        xown = np.ascontiguousarray(
            x_slot[c * SH:(c + 1) * SH].reshape(NW, 128, KIN)
            .transpose(1, 0, 2).reshape(128, NW * KIN))
        im = {
            "w_bf": w_bf, "b_vec": b, "a_vec": alpha,
            "dinv_own": per_core[c]["dinv"],
            "xown": xown,
            "idx16": per_core[c]["idx16"],
            "reldst": per_core[c]["reldst"],
        }
        for k in range(NQ):
            im[f"xq{k}"] = xq[k]
        in_maps.append(im)

    res = run_bass_kernel_spmd(nc, in_maps, core_ids=list(range(NCORES)))
    # out_pm[d, w*H+f] -> slot w*128+d; slot -> node via node_of_slot
    outs = []
    for c in range(NCORES):
        o = res.results[c]["out_pm"].astype(np.float32).reshape(128, NW, H).transpose(1, 0, 2)
        outs.append(o.reshape(SH, H))
    out_slots = np.concatenate(outs, axis=0)
    out = np.empty((N, H), np.float32)
    out[node_of_slot[real]] = out_slots[real]
    return out
